# revision 5
# baseline (speedup 1.0000x reference)
"""Binary linear layer (sign(X) @ sign(W) * scale) on 8 trn2 NeuronCores.

Strategy: data-parallel over the batch dim. Each core gets 1/8 of X's rows
(host-transposed so K lands on SBUF partitions), the full W, and
alpha/betta/gamma. On-device: sign() binarization on the ACT engine into a
low-precision dtype (exact for +-1), K-chunk-streamed matmul on the PE with
PSUM accumulation spans + an SBUF accumulator, scale grid computed on-device
(relu + PE outer product), fused scale multiply on the DVE, f32 output.
"""

import numpy as np

import concourse.bass as bass
import concourse.bacc as bacc
import concourse.mybir as mybir
from concourse.tile import TileContext

P = 128
B, IN, OUT = 8192, 4096, 2048
NCORES = 8


def build_kernel(
    M=B // NCORES,
    K=IN,
    N=OUT,
    mode="bf16",  # "bf16" | "fp8" (fp8 uses DoubleRow)
    span=8,  # k-subtiles accumulated per PSUM residency
    chunk=4,  # k-subtiles binarized per chunk tile
    fd=512,  # matmul moving free dim (psum tile width)
    stx_bufs=3,
    stw_bufs=2,
    xb_bufs=3,
    wb_bufs=3,
    acc_dt=mybir.dt.float32,
):
    f32 = mybir.dt.float32
    if mode == "bf16":
        bin_dt = mybir.dt.bfloat16
        kp = 1  # k-subtiles per matmul
        pm = None
    elif mode == "fp8":
        bin_dt = mybir.dt.float8e4
        kp = 2
        pm = mybir.MatmulPerfMode.DoubleRow
    else:
        raise ValueError(mode)

    ksub = K // P
    assert K % P == 0 and ksub % span == 0 and span % chunk == 0 and chunk % kp == 0
    nphase = ksub // span
    cpp = span // chunk  # chunks per phase
    nmt = M // P
    nnc = N // fd

    nc = bacc.Bacc("TRN2", debug=False, num_devices=NCORES)

    XT = nc.declare_dram_parameter("XT", [K, M], f32, isOutput=False)
    W = nc.declare_dram_parameter("W", [K, N], f32, isOutput=False)
    alpha = nc.declare_dram_parameter("alpha", [1], f32, isOutput=False)
    betta = nc.declare_dram_parameter("betta", [32], f32, isOutput=False)
    gamma = nc.declare_dram_parameter("gamma", [64], f32, isOutput=False)
    Y = nc.declare_dram_parameter("Y", [M, N], f32, isOutput=True)

    AF = mybir.ActivationFunctionType

    with TileContext(nc) as tc:
        with (
            tc.tile_pool(name="const", bufs=1) as cpool,
            tc.tile_pool(name="stage", bufs=2) as stpool,
            tc.tile_pool(name="bin", bufs=3) as binpool,
            tc.tile_pool(name="accp", bufs=1) as accpool,
            tc.tile_pool(name="outp", bufs=4) as outpool,
            tc.tile_pool(name="psum", bufs=8, space="PSUM") as pspool,
        ):
            # ---- scale grid: relu(alpha) * outer(relu(betta), relu(gamma)) ----
            a_t = cpool.tile([1, 1], f32, bufs=1)
            b_t = cpool.tile([1, 32], f32, bufs=1)
            g_t = cpool.tile([1, 64], f32, bufs=1)
            nc.sync.dma_start(out=a_t, in_=alpha[:])
            nc.sync.dma_start(out=b_t, in_=betta[:])
            nc.sync.dma_start(out=g_t, in_=gamma[:])
            nc.scalar.activation(a_t, a_t, AF.Relu)
            # relu(betta)*relu(alpha) == relu(betta*relu(alpha)) since relu(alpha)>=0
            b_s = cpool.tile([1, 32], f32, bufs=1)
            g_r = cpool.tile([1, 64], f32, bufs=1)
            nc.scalar.activation(b_s, b_t, AF.Relu, scale=a_t[0:1, 0:1])
            nc.scalar.activation(g_r, g_t, AF.Relu)
            ps_sc = pspool.tile([P, fd], f32, tag="mm", bufs=8)
            nc.tensor.matmul(ps_sc[:32, :64], lhsT=b_s, rhs=g_r, start=True, stop=True)
            scale_t = cpool.tile([P, N], f32, bufs=1)
            # flatten [32,64] -> row 0, then log-doubling partition broadcast
            sc_tmp = cpool.tile([32, 64], f32, bufs=1)
            nc.vector.tensor_copy(out=sc_tmp, in_=ps_sc[:32, :64])
            nc.sync.dma_start(out=scale_t[0:1, :], in_=sc_tmp)
            sh = 1
            while sh < P:
                nc.sync.dma_start(out=scale_t[sh : 2 * sh, :], in_=scale_t[0:sh, :])
                sh *= 2

            # ---- accumulator over phases ----
            if nphase > 1:
                acc = accpool.tile([P, nmt * N], acc_dt, bufs=1)

            # ---- main loop ----
            for ph in range(nphase):
                xbs, wbs = [], []
                for c in range(cpp):
                    xb = binpool.tile([P, chunk, M], bin_dt, tag="xb", bufs=xb_bufs)
                    wb = binpool.tile([P, chunk, N], bin_dt, tag="wb", bufs=wb_bufs)
                    for s in range(chunk):
                        ks = ph * span + c * chunk + s
                        xs = stpool.tile([P, M], f32, tag="xs", bufs=stx_bufs)
                        nc.sync.dma_start(out=xs, in_=XT[ks * P : (ks + 1) * P, :])
                        nc.scalar.activation(xb[:, s, :], xs, AF.Sign)
                        ws = stpool.tile([P, N], f32, tag="ws", bufs=stw_bufs)
                        nc.sync.dma_start(out=ws, in_=W[ks * P : (ks + 1) * P, :])
                        nc.scalar.activation(wb[:, s, :], ws, AF.Sign)
                    xbs.append(xb)
                    wbs.append(wb)

                for mt in range(nmt):
                    for nt in range(nnc):
                        ps = pspool.tile([P, fd], f32, tag="mm", bufs=8)
                        nmm = span // kp
                        i = 0
                        for c in range(cpp):
                            for s0 in range(0, chunk, kp):
                                lhs = (
                                    xbs[c][:, s0, mt * P : (mt + 1) * P]
                                    if kp == 1
                                    else xbs[c][:, s0 : s0 + kp, mt * P : (mt + 1) * P]
                                )
                                rhs = (
                                    wbs[c][:, s0, nt * fd : (nt + 1) * fd]
                                    if kp == 1
                                    else wbs[c][:, s0 : s0 + kp, nt * fd : (nt + 1) * fd]
                                )
                                nc.tensor.matmul(
                                    ps,
                                    lhsT=lhs,
                                    rhs=rhs,
                                    start=(i == 0),
                                    stop=(i == nmm - 1),
                                    perf_mode=pm,
                                )
                                i += 1
                        accsl = None
                        if nphase > 1:
                            accsl = acc[:, mt * N + nt * fd : mt * N + (nt + 1) * fd]
                        scsl = scale_t[:, nt * fd : (nt + 1) * fd]
                        if ph == 0 and nphase > 1:
                            nc.vector.tensor_copy(out=accsl, in_=ps)
                        elif ph < nphase - 1:
                            nc.vector.tensor_add(out=accsl, in0=accsl, in1=ps)
                        else:
                            ot = outpool.tile([P, fd], f32, tag="ot", bufs=4)
                            if nphase > 1:
                                nc.vector.tensor_add(out=ot, in0=accsl, in1=ps)
                                nc.vector.tensor_mul(out=ot, in0=ot, in1=scsl)
                            else:
                                nc.vector.tensor_mul(out=ot, in0=ps, in1=scsl)
                            nc.sync.dma_start(
                                out=Y[mt * P : (mt + 1) * P, nt * fd : (nt + 1) * fd],
                                in_=ot,
                            )
    return nc


_NC_CACHE = {}


def _get_nc(**kw):
    key = tuple(sorted(kw.items()))
    if key not in _NC_CACHE:
        nc = build_kernel(**kw)
        nc.finalize()  # runs the bacc passes (reg alloc etc.) pre-serialization
        _NC_CACHE[key] = nc
    return _NC_CACHE[key]


def _make_in_maps(X, W, alpha, betta, gamma):
    X = np.ascontiguousarray(np.asarray(X, dtype=np.float32))
    W = np.ascontiguousarray(np.asarray(W, dtype=np.float32))
    alpha = np.asarray(alpha, dtype=np.float32).reshape([1])
    betta = np.asarray(betta, dtype=np.float32).reshape([32])
    gamma = np.asarray(gamma, dtype=np.float32).reshape([64])
    rows = X.shape[0] // NCORES
    in_maps = []
    for c in range(NCORES):
        xt = np.ascontiguousarray(X[c * rows : (c + 1) * rows, :].T)
        in_maps.append(
            {"XT": xt, "W": W, "alpha": alpha, "betta": betta, "gamma": gamma}
        )
    return in_maps


def run_on_cores(inputs, trace=False, tmpdir=None, **build_kw):
    """Run the SPMD kernel on 8 cores; returns (Y_full, BassKernelResults)."""
    from concourse.bass_utils import run_bass_kernel_spmd

    nc = _get_nc(**build_kw)
    in_maps = _make_in_maps(**inputs)
    res = run_bass_kernel_spmd(
        nc, in_maps, list(range(NCORES)), trace=trace, tmpdir=tmpdir
    )
    Y = np.concatenate([r["Y"] for r in res.results], axis=0)
    return Y, res


def kernel(**inputs) -> np.ndarray:
    Y, _ = run_on_cores(inputs)
    return Y


# revision 11
# speedup vs baseline: 1.2541x; 1.2541x over previous
"""Binary linear layer (sign(X) @ sign(W) * scale) on 8 trn2 NeuronCores.

Strategy: data-parallel over the batch dim. Each core gets 1/8 of X's rows
(host-transposed so K lands on SBUF partitions), the full W, and
alpha/betta/gamma. On-device: sign() binarization on the ACT engine into a
low-precision dtype (exact for +-1), K-chunk-streamed matmul on the PE with
PSUM accumulation spans + an SBUF accumulator, scale grid computed on-device
(relu + PE outer product), fused scale multiply on the DVE, f32 output.
"""

import numpy as np

import concourse.bass as bass
import concourse.bacc as bacc
import concourse.mybir as mybir
from concourse.tile import TileContext

P = 128
B, IN, OUT = 8192, 4096, 2048
NCORES = 8


def build_kernel(
    M=B // NCORES,
    K=IN,
    N=OUT,
    mode="bf16",  # "bf16" | "fp8" (fp8 uses DoubleRow)
    span=8,  # k-subtiles accumulated per PSUM residency
    chunk=4,  # k-subtiles binarized per chunk tile
    fd=512,  # matmul moving free dim (psum tile width)
    stx_bufs=3,
    stw_bufs=2,
    xb_bufs=3,
    wb_bufs=3,
    acc_dt=mybir.dt.float32,
    upload_dt=mybir.dt.float32,  # XT/W transfer dtype (bf16 is sign-exact)
):
    f32 = mybir.dt.float32
    if mode == "bf16":
        bin_dt = mybir.dt.bfloat16
        kp = 1  # k-subtiles per matmul
        pm = None
    elif mode == "fp8":
        bin_dt = mybir.dt.float8e4
        kp = 2
        pm = mybir.MatmulPerfMode.DoubleRow
    else:
        raise ValueError(mode)

    ksub = K // P
    assert K % P == 0 and ksub % span == 0 and span % chunk == 0 and chunk % kp == 0
    nphase = ksub // span
    cpp = span // chunk  # chunks per phase
    nmt = M // P
    nnc = N // fd

    nc = bacc.Bacc("TRN2", debug=False, num_devices=NCORES)

    XT = nc.declare_dram_parameter("XT", [K, M], upload_dt, isOutput=False)
    W = nc.declare_dram_parameter("W", [K, N], upload_dt, isOutput=False)
    alpha = nc.declare_dram_parameter("alpha", [1], f32, isOutput=False)
    betta = nc.declare_dram_parameter("betta", [32], f32, isOutput=False)
    gamma = nc.declare_dram_parameter("gamma", [64], f32, isOutput=False)
    Y = nc.declare_dram_parameter("Y", [M, N], f32, isOutput=True)

    AF = mybir.ActivationFunctionType

    with TileContext(nc) as tc:
        with (
            tc.tile_pool(name="const", bufs=1) as cpool,
            tc.tile_pool(name="stage", bufs=2) as stpool,
            tc.tile_pool(name="bin", bufs=3) as binpool,
            tc.tile_pool(name="accp", bufs=1) as accpool,
            tc.tile_pool(name="outp", bufs=4) as outpool,
            tc.tile_pool(name="psum", bufs=8, space="PSUM") as pspool,
        ):
            # ---- scale grid: relu(alpha) * outer(relu(betta), relu(gamma)) ----
            a_t = cpool.tile([1, 1], f32, bufs=1)
            b_t = cpool.tile([1, 32], f32, bufs=1)
            g_t = cpool.tile([1, 64], f32, bufs=1)
            nc.sync.dma_start(out=a_t, in_=alpha[:])
            nc.sync.dma_start(out=b_t, in_=betta[:])
            nc.sync.dma_start(out=g_t, in_=gamma[:])
            nc.scalar.activation(a_t, a_t, AF.Relu)
            # relu(betta)*relu(alpha) == relu(betta*relu(alpha)) since relu(alpha)>=0
            b_s = cpool.tile([1, 32], f32, bufs=1)
            g_r = cpool.tile([1, 64], f32, bufs=1)
            nc.scalar.activation(b_s, b_t, AF.Relu, scale=a_t[0:1, 0:1])
            nc.scalar.activation(g_r, g_t, AF.Relu)
            ps_sc = pspool.tile([P, fd], f32, tag="mm", bufs=8)
            nc.tensor.matmul(ps_sc[:32, :64], lhsT=b_s, rhs=g_r, start=True, stop=True)
            scale_t = cpool.tile([P, N], f32, bufs=1)
            # flatten [32,64] -> row 0, then log-doubling partition broadcast
            sc_tmp = cpool.tile([32, 64], f32, bufs=1)
            nc.vector.tensor_copy(out=sc_tmp, in_=ps_sc[:32, :64])
            nc.sync.dma_start(out=scale_t[0:1, :], in_=sc_tmp)
            sh = 1
            while sh < P:
                nc.sync.dma_start(out=scale_t[sh : 2 * sh, :], in_=scale_t[0:sh, :])
                sh *= 2

            # ---- accumulator over phases ----
            if nphase > 1:
                acc = accpool.tile([P, nmt * N], acc_dt, bufs=1)

            # ---- main loop ----
            for ph in range(nphase):
                xbs, wbs = [], []
                for c in range(cpp):
                    xb = binpool.tile([P, chunk, M], bin_dt, tag="xb", bufs=xb_bufs)
                    wb = binpool.tile([P, chunk, N], bin_dt, tag="wb", bufs=wb_bufs)
                    for s in range(chunk):
                        ks = ph * span + c * chunk + s
                        xs = stpool.tile([P, M], upload_dt, tag="xs", bufs=stx_bufs)
                        nc.sync.dma_start(out=xs, in_=XT[ks * P : (ks + 1) * P, :])
                        nc.scalar.activation(xb[:, s, :], xs, AF.Sign)
                        ws = stpool.tile([P, N], upload_dt, tag="ws", bufs=stw_bufs)
                        nc.sync.dma_start(out=ws, in_=W[ks * P : (ks + 1) * P, :])
                        nc.scalar.activation(wb[:, s, :], ws, AF.Sign)
                    xbs.append(xb)
                    wbs.append(wb)

                for mt in range(nmt):
                    for nt in range(nnc):
                        ps = pspool.tile([P, fd], f32, tag="mm", bufs=8)
                        nmm = span // kp
                        i = 0
                        for c in range(cpp):
                            for s0 in range(0, chunk, kp):
                                lhs = (
                                    xbs[c][:, s0, mt * P : (mt + 1) * P]
                                    if kp == 1
                                    else xbs[c][:, s0 : s0 + kp, mt * P : (mt + 1) * P]
                                )
                                rhs = (
                                    wbs[c][:, s0, nt * fd : (nt + 1) * fd]
                                    if kp == 1
                                    else wbs[c][:, s0 : s0 + kp, nt * fd : (nt + 1) * fd]
                                )
                                nc.tensor.matmul(
                                    ps,
                                    lhsT=lhs,
                                    rhs=rhs,
                                    start=(i == 0),
                                    stop=(i == nmm - 1),
                                    perf_mode=pm,
                                )
                                i += 1
                        accsl = None
                        if nphase > 1:
                            accsl = acc[:, mt * N + nt * fd : mt * N + (nt + 1) * fd]
                        scsl = scale_t[:, nt * fd : (nt + 1) * fd]
                        if ph == 0 and nphase > 1:
                            nc.vector.tensor_copy(out=accsl, in_=ps)
                        elif ph < nphase - 1:
                            nc.vector.tensor_add(out=accsl, in0=accsl, in1=ps)
                        else:
                            ot = outpool.tile([P, fd], f32, tag="ot", bufs=4)
                            if nphase > 1:
                                nc.vector.tensor_add(out=ot, in0=accsl, in1=ps)
                                nc.vector.tensor_mul(out=ot, in0=ot, in1=scsl)
                            else:
                                nc.vector.tensor_mul(out=ot, in0=ps, in1=scsl)
                            nc.sync.dma_start(
                                out=Y[mt * P : (mt + 1) * P, nt * fd : (nt + 1) * fd],
                                in_=ot,
                            )
    return nc


_NC_CACHE = {}


def _get_nc(**kw):
    key = tuple(sorted(kw.items()))
    if key not in _NC_CACHE:
        nc = build_kernel(**kw)
        nc.finalize()  # runs the bacc passes (reg alloc etc.) pre-serialization
        _NC_CACHE[key] = nc
    return _NC_CACHE[key]


def _make_in_maps(X, W, alpha, betta, gamma, upload_dt=mybir.dt.float32):
    np_dt = mybir.dt.np(upload_dt)
    X = np.asarray(X, dtype=np.float32)
    # sign() is invariant under bf16 rounding, so a bf16 transfer dtype is
    # exact for this kernel's math
    W = np.ascontiguousarray(np.asarray(W, dtype=np.float32).astype(np_dt))
    alpha = np.asarray(alpha, dtype=np.float32).reshape([1])
    betta = np.asarray(betta, dtype=np.float32).reshape([32])
    gamma = np.asarray(gamma, dtype=np.float32).reshape([64])
    rows = X.shape[0] // NCORES
    in_maps = []
    for c in range(NCORES):
        xt = np.ascontiguousarray(X[c * rows : (c + 1) * rows, :].T.astype(np_dt))
        in_maps.append(
            {"XT": xt, "W": W, "alpha": alpha, "betta": betta, "gamma": gamma}
        )
    return in_maps


def run_on_cores(inputs, trace=False, tmpdir=None, **build_kw):
    """Run the SPMD kernel on 8 cores; returns (Y_full, BassKernelResults)."""
    from concourse.bass_utils import run_bass_kernel_spmd

    nc = _get_nc(**build_kw)
    in_maps = _make_in_maps(
        **inputs, upload_dt=build_kw.get("upload_dt", mybir.dt.float32)
    )
    res = run_bass_kernel_spmd(
        nc, in_maps, list(range(NCORES)), trace=trace, tmpdir=tmpdir
    )
    Y = np.concatenate([r["Y"] for r in res.results], axis=0)
    return Y, res


PROD_KW = dict(
    mode="fp8",
    span=16,
    chunk=8,
    upload_dt=mybir.dt.bfloat16,
    stx_bufs=4,
    stw_bufs=3,
    xb_bufs=3,
    wb_bufs=3,
)


def kernel(**inputs) -> np.ndarray:
    Y, _ = run_on_cores(inputs, **PROD_KW)
    return Y


# revision 16
# speedup vs baseline: 1.2893x; 1.0280x over previous
"""Binary linear layer (sign(X) @ sign(W) * scale) on 8 trn2 NeuronCores.

Strategy: data-parallel over the batch dim. Each core gets 1/8 of X's rows
(host-transposed so K lands on SBUF partitions), the full W, and
alpha/betta/gamma. On-device: sign() binarization on the ACT engine into a
low-precision dtype (exact for +-1), K-chunk-streamed matmul on the PE with
PSUM accumulation spans + an SBUF accumulator, scale grid computed on-device
(relu + PE outer product), fused scale multiply on the DVE, f32 output.
"""

import numpy as np

import concourse.bass as bass
import concourse.bacc as bacc
import concourse.mybir as mybir
from concourse.tile import TileContext

P = 128
B, IN, OUT = 8192, 4096, 2048
NCORES = 8


def build_kernel(
    M=B // NCORES,
    K=IN,
    N=OUT,
    mode="bf16",  # "bf16" | "fp8" (fp8 uses DoubleRow)
    span=8,  # k-subtiles accumulated per PSUM residency
    chunk=4,  # k-subtiles binarized per chunk tile
    fd=512,  # matmul moving free dim (psum tile width)
    stx_bufs=3,
    stw_bufs=2,
    xb_bufs=3,
    wb_bufs=3,
    acc_dt=mybir.dt.float32,
    upload_dt=mybir.dt.float32,  # XT/W transfer dtype (bf16 is sign-exact)
    x_on_dve=False,  # binarize X on DVE as +-0.5 (scale x2), W on ACT as +-1
    copy_on_act=False,  # phase-0 PSUM eviction on ACT instead of DVE
):
    f32 = mybir.dt.float32
    if mode == "bf16":
        bin_dt = mybir.dt.bfloat16
        kp = 1  # k-subtiles per matmul
        pm = None
    elif mode == "fp8":
        bin_dt = mybir.dt.float8e4
        kp = 2
        pm = mybir.MatmulPerfMode.DoubleRow
    else:
        raise ValueError(mode)

    ksub = K // P
    assert K % P == 0 and ksub % span == 0 and span % chunk == 0 and chunk % kp == 0
    nphase = ksub // span
    cpp = span // chunk  # chunks per phase
    nmt = M // P
    nnc = N // fd

    nc = bacc.Bacc("TRN2", debug=False, num_devices=NCORES)

    XT = nc.declare_dram_parameter("XT", [K, M], upload_dt, isOutput=False)
    W = nc.declare_dram_parameter("W", [K, N], upload_dt, isOutput=False)
    alpha = nc.declare_dram_parameter("alpha", [1], f32, isOutput=False)
    betta = nc.declare_dram_parameter("betta", [32], f32, isOutput=False)
    gamma = nc.declare_dram_parameter("gamma", [64], f32, isOutput=False)
    Y = nc.declare_dram_parameter("Y", [M, N], f32, isOutput=True)

    AF = mybir.ActivationFunctionType

    with TileContext(nc) as tc:
        with (
            tc.tile_pool(name="const", bufs=1) as cpool,
            tc.tile_pool(name="stage", bufs=2) as stpool,
            tc.tile_pool(name="bin", bufs=3) as binpool,
            tc.tile_pool(name="accp", bufs=1) as accpool,
            tc.tile_pool(name="outp", bufs=4) as outpool,
            tc.tile_pool(name="psum", bufs=8, space="PSUM") as pspool,
        ):
            # ---- scale grid: relu(alpha) * outer(relu(betta), relu(gamma)) ----
            a_t = cpool.tile([1, 1], f32, bufs=1)
            b_t = cpool.tile([1, 32], f32, bufs=1)
            g_t = cpool.tile([1, 64], f32, bufs=1)
            nc.sync.dma_start(out=a_t, in_=alpha[:])
            nc.sync.dma_start(out=b_t, in_=betta[:])
            nc.sync.dma_start(out=g_t, in_=gamma[:])
            # with the +-0.5 X encoding every product carries a 0.5 factor;
            # fold the compensating 2x into relu(alpha)
            nc.scalar.activation(a_t, a_t, AF.Relu, scale=2.0 if x_on_dve else 1.0)
            # relu(betta)*relu(alpha) == relu(betta*relu(alpha)) since relu(alpha)>=0
            b_s = cpool.tile([1, 32], f32, bufs=1)
            g_r = cpool.tile([1, 64], f32, bufs=1)
            nc.scalar.activation(b_s, b_t, AF.Relu, scale=a_t[0:1, 0:1])
            nc.scalar.activation(g_r, g_t, AF.Relu)
            ps_sc = pspool.tile([P, fd], f32, tag="mm", bufs=8)
            nc.tensor.matmul(ps_sc[:32, :64], lhsT=b_s, rhs=g_r, start=True, stop=True)
            scale_t = cpool.tile([P, N], f32, bufs=1)
            # flatten [32,64] -> row 0, then log-doubling partition broadcast
            sc_tmp = cpool.tile([32, 64], f32, bufs=1)
            nc.vector.tensor_copy(out=sc_tmp, in_=ps_sc[:32, :64])
            nc.sync.dma_start(out=scale_t[0:1, :], in_=sc_tmp)
            sh = 1
            while sh < P:
                nc.sync.dma_start(out=scale_t[sh : 2 * sh, :], in_=scale_t[0:sh, :])
                sh *= 2

            # ---- accumulator over phases ----
            if nphase > 1:
                acc = accpool.tile([P, nmt * N], acc_dt, bufs=1)

            # ---- main loop ----
            for ph in range(nphase):
                xbs, wbs = [], []
                for c in range(cpp):
                    xb = binpool.tile([P, chunk, M], bin_dt, tag="xb", bufs=xb_bufs)
                    wb = binpool.tile([P, chunk, N], bin_dt, tag="wb", bufs=wb_bufs)
                    for s in range(chunk):
                        ks = ph * span + c * chunk + s
                        xs = stpool.tile([P, M], upload_dt, tag="xs", bufs=stx_bufs)
                        nc.sync.dma_start(out=xs, in_=XT[ks * P : (ks + 1) * P, :])
                        if x_on_dve:
                            nc.vector.tensor_scalar(
                                out=xb[:, s, :],
                                in0=xs,
                                scalar1=0.0,
                                scalar2=0.5,
                                op0=mybir.AluOpType.is_ge,
                                op1=mybir.AluOpType.subtract,
                            )
                        else:
                            nc.scalar.activation(xb[:, s, :], xs, AF.Sign)
                        ws = stpool.tile([P, N], upload_dt, tag="ws", bufs=stw_bufs)
                        nc.sync.dma_start(out=ws, in_=W[ks * P : (ks + 1) * P, :])
                        nc.scalar.activation(wb[:, s, :], ws, AF.Sign)
                    xbs.append(xb)
                    wbs.append(wb)

                for mt in range(nmt):
                    for nt in range(nnc):
                        ps = pspool.tile([P, fd], f32, tag="mm", bufs=8)
                        nmm = span // kp
                        i = 0
                        for c in range(cpp):
                            for s0 in range(0, chunk, kp):
                                lhs = (
                                    xbs[c][:, s0, mt * P : (mt + 1) * P]
                                    if kp == 1
                                    else xbs[c][:, s0 : s0 + kp, mt * P : (mt + 1) * P]
                                )
                                rhs = (
                                    wbs[c][:, s0, nt * fd : (nt + 1) * fd]
                                    if kp == 1
                                    else wbs[c][:, s0 : s0 + kp, nt * fd : (nt + 1) * fd]
                                )
                                nc.tensor.matmul(
                                    ps,
                                    lhsT=lhs,
                                    rhs=rhs,
                                    start=(i == 0),
                                    stop=(i == nmm - 1),
                                    perf_mode=pm,
                                )
                                i += 1
                        accsl = None
                        if nphase > 1:
                            accsl = acc[:, mt * N + nt * fd : mt * N + (nt + 1) * fd]
                        scsl = scale_t[:, nt * fd : (nt + 1) * fd]
                        if ph == 0 and nphase > 1:
                            if copy_on_act:
                                nc.scalar.copy(accsl, ps)
                            else:
                                nc.vector.tensor_copy(out=accsl, in_=ps)
                        elif ph < nphase - 1:
                            nc.vector.tensor_add(out=accsl, in0=accsl, in1=ps)
                        else:
                            ot = outpool.tile([P, fd], f32, tag="ot", bufs=4)
                            if nphase > 1:
                                nc.vector.tensor_add(out=ot, in0=accsl, in1=ps)
                                nc.vector.tensor_mul(out=ot, in0=ot, in1=scsl)
                            else:
                                nc.vector.tensor_mul(out=ot, in0=ps, in1=scsl)
                            nc.sync.dma_start(
                                out=Y[mt * P : (mt + 1) * P, nt * fd : (nt + 1) * fd],
                                in_=ot,
                            )
    return nc


_NC_CACHE = {}


def _get_nc(**kw):
    key = tuple(sorted(kw.items()))
    if key not in _NC_CACHE:
        nc = build_kernel(**kw)
        nc.finalize()  # runs the bacc passes (reg alloc etc.) pre-serialization
        _NC_CACHE[key] = nc
    return _NC_CACHE[key]


def _make_in_maps(X, W, alpha, betta, gamma, upload_dt=mybir.dt.float32):
    np_dt = mybir.dt.np(upload_dt)
    X = np.asarray(X, dtype=np.float32)
    # sign() is invariant under bf16 rounding, so a bf16 transfer dtype is
    # exact for this kernel's math
    W = np.ascontiguousarray(np.asarray(W, dtype=np.float32).astype(np_dt))
    alpha = np.asarray(alpha, dtype=np.float32).reshape([1])
    betta = np.asarray(betta, dtype=np.float32).reshape([32])
    gamma = np.asarray(gamma, dtype=np.float32).reshape([64])
    rows = X.shape[0] // NCORES
    in_maps = []
    for c in range(NCORES):
        xt = np.ascontiguousarray(X[c * rows : (c + 1) * rows, :].T.astype(np_dt))
        in_maps.append(
            {"XT": xt, "W": W, "alpha": alpha, "betta": betta, "gamma": gamma}
        )
    return in_maps


def run_on_cores(inputs, trace=False, tmpdir=None, **build_kw):
    """Run the SPMD kernel on 8 cores; returns (Y_full, BassKernelResults)."""
    from concourse.bass_utils import run_bass_kernel_spmd

    nc = _get_nc(**build_kw)
    in_maps = _make_in_maps(
        **inputs, upload_dt=build_kw.get("upload_dt", mybir.dt.float32)
    )
    res = run_bass_kernel_spmd(
        nc, in_maps, list(range(NCORES)), trace=trace, tmpdir=tmpdir
    )
    Y = np.concatenate([r["Y"] for r in res.results], axis=0)
    return Y, res


PROD_KW = dict(
    mode="fp8",
    span=16,
    chunk=4,
    upload_dt=mybir.dt.bfloat16,
    stx_bufs=6,
    stw_bufs=4,
    xb_bufs=6,
    wb_bufs=6,
    acc_dt=mybir.dt.float16,
    x_on_dve=True,
    copy_on_act=True,
)


def kernel(**inputs) -> np.ndarray:
    Y, _ = run_on_cores(inputs, **PROD_KW)
    return Y


# revision 22
# speedup vs baseline: 1.3897x; 1.0779x over previous
"""Binary linear layer (sign(X) @ sign(W) * scale) on 8 trn2 NeuronCores.

Strategy: data-parallel over the batch dim. Each core gets 1/8 of X's rows
(host-transposed so K lands on SBUF partitions), the full W, and
alpha/betta/gamma. On-device: sign() binarization on the ACT engine into a
low-precision dtype (exact for +-1), K-chunk-streamed matmul on the PE with
PSUM accumulation spans + an SBUF accumulator, scale grid computed on-device
(relu + PE outer product), fused scale multiply on the DVE, f32 output.
"""

import numpy as np

import concourse.bass as bass
import concourse.bacc as bacc
import concourse.mybir as mybir
from concourse.tile import TileContext

P = 128
B, IN, OUT = 8192, 4096, 2048
NCORES = 8


def build_kernel(
    M=B // NCORES,
    K=IN,
    N=OUT,
    mode="bf16",  # "bf16" | "fp8" (fp8 uses DoubleRow)
    span=8,  # k-subtiles per PSUM residency: int, or tuple of per-phase spans
    chunk=4,  # k-subtiles binarized per chunk tile
    fd=512,  # matmul moving free dim (psum tile width)
    stx_bufs=3,
    stw_bufs=2,
    xb_bufs=3,
    wb_bufs=3,
    acc_dt=mybir.dt.float32,
    upload_dt=mybir.dt.float32,  # XT/W transfer dtype (bf16 is sign-exact)
    x_on_dve=False,  # binarize X on DVE as +-0.5 (scale x2), W on ACT as +-1
    copy_on_act=False,  # phase-0 PSUM eviction on ACT instead of DVE
):
    f32 = mybir.dt.float32
    if mode == "bf16":
        bin_dt = mybir.dt.bfloat16
        kp = 1  # k-subtiles per matmul
        pm = None
    elif mode == "fp8":
        bin_dt = mybir.dt.float8e4
        kp = 2
        pm = mybir.MatmulPerfMode.DoubleRow
    else:
        raise ValueError(mode)

    ksub = K // P
    if isinstance(span, int):
        assert ksub % span == 0
        spans = [span] * (ksub // span)
    else:
        spans = list(span)
    assert K % P == 0 and sum(spans) == ksub and chunk % kp == 0
    assert all(s % chunk == 0 for s in spans)
    nphase = len(spans)
    nmt = M // P
    nnc = N // fd

    nc = bacc.Bacc("TRN2", debug=False, num_devices=NCORES)

    XT = nc.declare_dram_parameter("XT", [K, M], upload_dt, isOutput=False)
    W = nc.declare_dram_parameter("W", [K, N], upload_dt, isOutput=False)
    alpha = nc.declare_dram_parameter("alpha", [1], f32, isOutput=False)
    betta = nc.declare_dram_parameter("betta", [32], f32, isOutput=False)
    gamma = nc.declare_dram_parameter("gamma", [64], f32, isOutput=False)
    Y = nc.declare_dram_parameter("Y", [M, N], f32, isOutput=True)

    AF = mybir.ActivationFunctionType

    with TileContext(nc) as tc:
        with (
            tc.tile_pool(name="const", bufs=1) as cpool,
            tc.tile_pool(name="stage", bufs=2) as stpool,
            tc.tile_pool(name="bin", bufs=3) as binpool,
            tc.tile_pool(name="accp", bufs=1) as accpool,
            tc.tile_pool(name="outp", bufs=4) as outpool,
            tc.tile_pool(name="psum", bufs=8, space="PSUM") as pspool,
        ):
            # ---- scale grid: relu(alpha) * outer(relu(betta), relu(gamma)) ----
            a_t = cpool.tile([1, 1], f32, bufs=1)
            b_t = cpool.tile([1, 32], f32, bufs=1)
            g_t = cpool.tile([1, 64], f32, bufs=1)
            nc.sync.dma_start(out=a_t, in_=alpha[:])
            nc.sync.dma_start(out=b_t, in_=betta[:])
            nc.sync.dma_start(out=g_t, in_=gamma[:])
            # with the +-0.5 X encoding every product carries a 0.5 factor;
            # fold the compensating 2x into relu(alpha)
            nc.scalar.activation(a_t, a_t, AF.Relu, scale=2.0 if x_on_dve else 1.0)
            # relu(betta)*relu(alpha) == relu(betta*relu(alpha)) since relu(alpha)>=0
            b_s = cpool.tile([1, 32], f32, bufs=1)
            g_r = cpool.tile([1, 64], f32, bufs=1)
            nc.scalar.activation(b_s, b_t, AF.Relu, scale=a_t[0:1, 0:1])
            nc.scalar.activation(g_r, g_t, AF.Relu)
            ps_sc = pspool.tile([P, fd], f32, tag="mm", bufs=8)
            nc.tensor.matmul(ps_sc[:32, :64], lhsT=b_s, rhs=g_r, start=True, stop=True)
            scale_t = cpool.tile([P, N], f32, bufs=1)
            # flatten [32,64] -> row 0, then log-doubling partition broadcast
            sc_tmp = cpool.tile([32, 64], f32, bufs=1)
            nc.vector.tensor_copy(out=sc_tmp, in_=ps_sc[:32, :64])
            nc.sync.dma_start(out=scale_t[0:1, :], in_=sc_tmp)
            sh = 1
            while sh < P:
                nc.sync.dma_start(out=scale_t[sh : 2 * sh, :], in_=scale_t[0:sh, :])
                sh *= 2

            # ---- accumulator over phases ----
            if nphase > 1:
                acc = accpool.tile([P, nmt * N], acc_dt, bufs=1)

            # ---- main loop ----
            ph_base = 0
            for ph in range(nphase):
                span_p = spans[ph]
                cpp = span_p // chunk
                xbs, wbs = [], []
                for c in range(cpp):
                    xb = binpool.tile([P, chunk, M], bin_dt, tag="xb", bufs=xb_bufs)
                    wb = binpool.tile([P, chunk, N], bin_dt, tag="wb", bufs=wb_bufs)
                    for s in range(chunk):
                        ks = ph_base + c * chunk + s
                        xs = stpool.tile([P, M], upload_dt, tag="xs", bufs=stx_bufs)
                        nc.sync.dma_start(out=xs, in_=XT[ks * P : (ks + 1) * P, :])
                        if x_on_dve:
                            nc.vector.tensor_scalar(
                                out=xb[:, s, :],
                                in0=xs,
                                scalar1=0.0,
                                scalar2=0.5,
                                op0=mybir.AluOpType.is_ge,
                                op1=mybir.AluOpType.subtract,
                            )
                        else:
                            nc.scalar.activation(xb[:, s, :], xs, AF.Sign)
                        ws = stpool.tile([P, N], upload_dt, tag="ws", bufs=stw_bufs)
                        nc.sync.dma_start(out=ws, in_=W[ks * P : (ks + 1) * P, :])
                        nc.scalar.activation(wb[:, s, :], ws, AF.Sign)
                    xbs.append(xb)
                    wbs.append(wb)

                for mt in range(nmt):
                    for nt in range(nnc):
                        ps = pspool.tile([P, fd], f32, tag="mm", bufs=8)
                        nmm = span_p // kp
                        i = 0
                        for c in range(cpp):
                            for s0 in range(0, chunk, kp):
                                lhs = (
                                    xbs[c][:, s0, mt * P : (mt + 1) * P]
                                    if kp == 1
                                    else xbs[c][:, s0 : s0 + kp, mt * P : (mt + 1) * P]
                                )
                                rhs = (
                                    wbs[c][:, s0, nt * fd : (nt + 1) * fd]
                                    if kp == 1
                                    else wbs[c][:, s0 : s0 + kp, nt * fd : (nt + 1) * fd]
                                )
                                nc.tensor.matmul(
                                    ps,
                                    lhsT=lhs,
                                    rhs=rhs,
                                    start=(i == 0),
                                    stop=(i == nmm - 1),
                                    perf_mode=pm,
                                )
                                i += 1
                        accsl = None
                        if nphase > 1:
                            accsl = acc[:, mt * N + nt * fd : mt * N + (nt + 1) * fd]
                        scsl = scale_t[:, nt * fd : (nt + 1) * fd]
                        if ph == 0 and nphase > 1:
                            if copy_on_act:
                                nc.scalar.copy(accsl, ps)
                            else:
                                nc.vector.tensor_copy(out=accsl, in_=ps)
                        elif ph < nphase - 1:
                            nc.vector.tensor_add(out=accsl, in0=accsl, in1=ps)
                        else:
                            ot = outpool.tile([P, fd], f32, tag="ot", bufs=4)
                            if nphase > 1:
                                nc.vector.tensor_add(out=ot, in0=accsl, in1=ps)
                                nc.vector.tensor_mul(out=ot, in0=ot, in1=scsl)
                            else:
                                nc.vector.tensor_mul(out=ot, in0=ps, in1=scsl)
                            nc.sync.dma_start(
                                out=Y[mt * P : (mt + 1) * P, nt * fd : (nt + 1) * fd],
                                in_=ot,
                            )
                ph_base += span_p
    return nc


_NC_CACHE = {}


def _get_nc(**kw):
    key = tuple(sorted(kw.items()))
    if key not in _NC_CACHE:
        nc = build_kernel(**kw)
        nc.finalize()  # runs the bacc passes (reg alloc etc.) pre-serialization
        _NC_CACHE[key] = nc
    return _NC_CACHE[key]


def _make_in_maps(X, W, alpha, betta, gamma, upload_dt=mybir.dt.float32):
    np_dt = mybir.dt.np(upload_dt)
    X = np.asarray(X, dtype=np.float32)
    # sign() is invariant under bf16 rounding, so a bf16 transfer dtype is
    # exact for this kernel's math
    W = np.ascontiguousarray(np.asarray(W, dtype=np.float32).astype(np_dt))
    alpha = np.asarray(alpha, dtype=np.float32).reshape([1])
    betta = np.asarray(betta, dtype=np.float32).reshape([32])
    gamma = np.asarray(gamma, dtype=np.float32).reshape([64])
    rows = X.shape[0] // NCORES
    in_maps = []
    for c in range(NCORES):
        xt = np.ascontiguousarray(X[c * rows : (c + 1) * rows, :].T.astype(np_dt))
        in_maps.append(
            {"XT": xt, "W": W, "alpha": alpha, "betta": betta, "gamma": gamma}
        )
    return in_maps


def run_on_cores(inputs, trace=False, tmpdir=None, **build_kw):
    """Run the SPMD kernel on 8 cores; returns (Y_full, BassKernelResults)."""
    from concourse.bass_utils import run_bass_kernel_spmd

    nc = _get_nc(**build_kw)
    in_maps = _make_in_maps(
        **inputs, upload_dt=build_kw.get("upload_dt", mybir.dt.float32)
    )
    res = run_bass_kernel_spmd(
        nc, in_maps, list(range(NCORES)), trace=trace, tmpdir=tmpdir
    )
    Y = np.concatenate([r["Y"] for r in res.results], axis=0)
    return Y, res


PROD_KW = dict(
    mode="fp8",
    span=(8, 8, 16),
    chunk=2,
    upload_dt=mybir.dt.bfloat16,
    stx_bufs=6,
    stw_bufs=4,
    xb_bufs=16,  # whole-K resident: DMA free-runs ahead of the PE
    wb_bufs=16,
    acc_dt=mybir.dt.float16,
    x_on_dve=True,
    copy_on_act=True,
)


def kernel(**inputs) -> np.ndarray:
    Y, _ = run_on_cores(inputs, **PROD_KW)
    return Y


# revision 26
# speedup vs baseline: 1.3924x; 1.0020x over previous
"""Binary linear layer (sign(X) @ sign(W) * scale) on 8 trn2 NeuronCores.

Strategy: data-parallel over the batch dim. Each core gets 1/8 of X's rows
(host-transposed so K lands on SBUF partitions), the full W, and
alpha/betta/gamma. On-device: sign() binarization on the ACT engine into a
low-precision dtype (exact for +-1), K-chunk-streamed matmul on the PE with
PSUM accumulation spans + an SBUF accumulator, scale grid computed on-device
(relu + PE outer product), fused scale multiply on the DVE, f32 output.
"""

import numpy as np

import concourse.bass as bass
import concourse.bacc as bacc
import concourse.mybir as mybir
from concourse.tile import TileContext

P = 128
B, IN, OUT = 8192, 4096, 2048
NCORES = 8


def build_kernel(
    M=B // NCORES,
    K=IN,
    N=OUT,
    mode="bf16",  # "bf16" | "fp8" (fp8 uses DoubleRow)
    span=8,  # k-subtiles per PSUM residency: int, or tuple of per-phase spans
    chunk=4,  # k-subtiles binarized per chunk tile
    fd=512,  # matmul moving free dim (psum tile width)
    stx_bufs=3,
    stw_bufs=2,
    xb_bufs=3,
    wb_bufs=3,
    acc_dt=mybir.dt.float32,
    upload_dt=mybir.dt.float32,  # XT/W transfer dtype (bf16 is sign-exact)
    x_on_dve=False,  # binarize X on DVE as +-0.5 (scale x2), W on ACT as +-1
    copy_on_act=False,  # phase-0 PSUM eviction on ACT instead of DVE
    w_split=0,  # if >0: W cols [w_split:] binarized on DVE as +-0.5 (scale x2)
):
    f32 = mybir.dt.float32
    if mode == "bf16":
        bin_dt = mybir.dt.bfloat16
        kp = 1  # k-subtiles per matmul
        pm = None
    elif mode == "fp8":
        bin_dt = mybir.dt.float8e4
        kp = 2
        pm = mybir.MatmulPerfMode.DoubleRow
    else:
        raise ValueError(mode)

    ksub = K // P
    if isinstance(span, int):
        assert ksub % span == 0
        spans = [span] * (ksub // span)
    else:
        spans = list(span)
    assert K % P == 0 and sum(spans) == ksub and chunk % kp == 0
    assert all(s % chunk == 0 for s in spans)
    nphase = len(spans)
    nmt = M // P
    nnc = N // fd

    nc = bacc.Bacc("TRN2", debug=False, num_devices=NCORES)

    XT = nc.declare_dram_parameter("XT", [K, M], upload_dt, isOutput=False)
    W = nc.declare_dram_parameter("W", [K, N], upload_dt, isOutput=False)
    alpha = nc.declare_dram_parameter("alpha", [1], f32, isOutput=False)
    betta = nc.declare_dram_parameter("betta", [32], f32, isOutput=False)
    gamma = nc.declare_dram_parameter("gamma", [64], f32, isOutput=False)
    Y = nc.declare_dram_parameter("Y", [M, N], f32, isOutput=True)

    AF = mybir.ActivationFunctionType

    with TileContext(nc) as tc:
        with (
            tc.tile_pool(name="const", bufs=1) as cpool,
            tc.tile_pool(name="stage", bufs=2) as stpool,
            tc.tile_pool(name="bin", bufs=3) as binpool,
            tc.tile_pool(name="accp", bufs=1) as accpool,
            tc.tile_pool(name="outp", bufs=4) as outpool,
            tc.tile_pool(name="psum", bufs=8, space="PSUM") as pspool,
        ):
            # ---- scale grid: relu(alpha) * outer(relu(betta), relu(gamma)) ----
            a_t = cpool.tile([1, 1], f32, bufs=1)
            b_t = cpool.tile([1, 32], f32, bufs=1)
            g_t = cpool.tile([1, 64], f32, bufs=1)
            nc.sync.dma_start(out=a_t, in_=alpha[:])
            nc.sync.dma_start(out=b_t, in_=betta[:])
            nc.sync.dma_start(out=g_t, in_=gamma[:])
            # with the +-0.5 X encoding every product carries a 0.5 factor;
            # fold the compensating 2x into relu(alpha)
            nc.scalar.activation(a_t, a_t, AF.Relu, scale=2.0 if x_on_dve else 1.0)
            # relu(betta)*relu(alpha) == relu(betta*relu(alpha)) since relu(alpha)>=0
            b_s = cpool.tile([1, 32], f32, bufs=1)
            g_r = cpool.tile([1, 64], f32, bufs=1)
            nc.scalar.activation(b_s, b_t, AF.Relu, scale=a_t[0:1, 0:1])
            nc.scalar.activation(g_r, g_t, AF.Relu)
            ps_sc = pspool.tile([P, fd], f32, tag="mm", bufs=8)
            nc.tensor.matmul(ps_sc[:32, :64], lhsT=b_s, rhs=g_r, start=True, stop=True)
            scale_t = cpool.tile([P, N], f32, bufs=1)
            # flatten [32,64] -> row 0, then log-doubling partition broadcast
            sc_tmp = cpool.tile([32, 64], f32, bufs=1)
            nc.vector.tensor_copy(out=sc_tmp, in_=ps_sc[:32, :64])
            nc.sync.dma_start(out=scale_t[0:1, :], in_=sc_tmp)
            sh = 1
            while sh < P:
                nc.sync.dma_start(out=scale_t[sh : 2 * sh, :], in_=scale_t[0:sh, :])
                sh *= 2
            if w_split:
                # W cols [w_split:] use the +-0.5 encoding -> 2x those columns
                nc.vector.tensor_scalar_mul(
                    out=scale_t[:, w_split:], in0=scale_t[:, w_split:], scalar1=2.0
                )

            # ---- accumulator over phases ----
            if nphase > 1:
                acc = accpool.tile([P, nmt * N], acc_dt, bufs=1)

            # ---- main loop ----
            ph_base = 0
            for ph in range(nphase):
                span_p = spans[ph]
                cpp = span_p // chunk
                xbs, wbs = [], []
                for c in range(cpp):
                    xb = binpool.tile([P, chunk, M], bin_dt, tag="xb", bufs=xb_bufs)
                    wb = binpool.tile([P, chunk, N], bin_dt, tag="wb", bufs=wb_bufs)
                    for s in range(chunk):
                        ks = ph_base + c * chunk + s
                        xs = stpool.tile([P, M], upload_dt, tag="xs", bufs=stx_bufs)
                        nc.sync.dma_start(out=xs, in_=XT[ks * P : (ks + 1) * P, :])
                        if x_on_dve:
                            nc.vector.tensor_scalar(
                                out=xb[:, s, :],
                                in0=xs,
                                scalar1=0.0,
                                scalar2=0.5,
                                op0=mybir.AluOpType.is_ge,
                                op1=mybir.AluOpType.subtract,
                            )
                        else:
                            nc.scalar.activation(xb[:, s, :], xs, AF.Sign)
                        ws = stpool.tile([P, N], upload_dt, tag="ws", bufs=stw_bufs)
                        nc.sync.dma_start(out=ws, in_=W[ks * P : (ks + 1) * P, :])
                        if w_split:
                            nc.scalar.activation(
                                wb[:, s, :w_split], ws[:, :w_split], AF.Sign
                            )
                            nc.vector.tensor_scalar(
                                out=wb[:, s, w_split:],
                                in0=ws[:, w_split:],
                                scalar1=0.0,
                                scalar2=0.5,
                                op0=mybir.AluOpType.is_ge,
                                op1=mybir.AluOpType.subtract,
                            )
                        else:
                            nc.scalar.activation(wb[:, s, :], ws, AF.Sign)
                    xbs.append(xb)
                    wbs.append(wb)

                for mt in range(nmt):
                    for nt in range(nnc):
                        ps = pspool.tile([P, fd], f32, tag="mm", bufs=8)
                        nmm = span_p // kp
                        i = 0
                        for c in range(cpp):
                            for s0 in range(0, chunk, kp):
                                lhs = (
                                    xbs[c][:, s0, mt * P : (mt + 1) * P]
                                    if kp == 1
                                    else xbs[c][:, s0 : s0 + kp, mt * P : (mt + 1) * P]
                                )
                                rhs = (
                                    wbs[c][:, s0, nt * fd : (nt + 1) * fd]
                                    if kp == 1
                                    else wbs[c][:, s0 : s0 + kp, nt * fd : (nt + 1) * fd]
                                )
                                nc.tensor.matmul(
                                    ps,
                                    lhsT=lhs,
                                    rhs=rhs,
                                    start=(i == 0),
                                    stop=(i == nmm - 1),
                                    perf_mode=pm,
                                )
                                i += 1
                        accsl = None
                        if nphase > 1:
                            accsl = acc[:, mt * N + nt * fd : mt * N + (nt + 1) * fd]
                        scsl = scale_t[:, nt * fd : (nt + 1) * fd]
                        if ph == 0 and nphase > 1:
                            if copy_on_act:
                                nc.scalar.copy(accsl, ps)
                            else:
                                nc.vector.tensor_copy(out=accsl, in_=ps)
                        elif ph < nphase - 1:
                            nc.vector.tensor_add(out=accsl, in0=accsl, in1=ps)
                        else:
                            ot = outpool.tile([P, fd], f32, tag="ot", bufs=4)
                            if nphase > 1:
                                nc.vector.tensor_add(out=ot, in0=accsl, in1=ps)
                                nc.vector.tensor_mul(out=ot, in0=ot, in1=scsl)
                            else:
                                nc.vector.tensor_mul(out=ot, in0=ps, in1=scsl)
                            nc.sync.dma_start(
                                out=Y[mt * P : (mt + 1) * P, nt * fd : (nt + 1) * fd],
                                in_=ot,
                            )
                ph_base += span_p
    return nc


_NC_CACHE = {}


def _get_nc(**kw):
    key = tuple(sorted(kw.items()))
    if key not in _NC_CACHE:
        nc = build_kernel(**kw)
        nc.finalize()  # runs the bacc passes (reg alloc etc.) pre-serialization
        _NC_CACHE[key] = nc
    return _NC_CACHE[key]


def _make_in_maps(X, W, alpha, betta, gamma, upload_dt=mybir.dt.float32):
    np_dt = mybir.dt.np(upload_dt)
    X = np.asarray(X, dtype=np.float32)
    # sign() is invariant under bf16 rounding, so a bf16 transfer dtype is
    # exact for this kernel's math
    W = np.ascontiguousarray(np.asarray(W, dtype=np.float32).astype(np_dt))
    alpha = np.asarray(alpha, dtype=np.float32).reshape([1])
    betta = np.asarray(betta, dtype=np.float32).reshape([32])
    gamma = np.asarray(gamma, dtype=np.float32).reshape([64])
    rows = X.shape[0] // NCORES
    in_maps = []
    for c in range(NCORES):
        xt = np.ascontiguousarray(X[c * rows : (c + 1) * rows, :].T.astype(np_dt))
        in_maps.append(
            {"XT": xt, "W": W, "alpha": alpha, "betta": betta, "gamma": gamma}
        )
    return in_maps


def run_on_cores(inputs, trace=False, tmpdir=None, **build_kw):
    """Run the SPMD kernel on 8 cores; returns (Y_full, BassKernelResults)."""
    from concourse.bass_utils import run_bass_kernel_spmd

    nc = _get_nc(**build_kw)
    in_maps = _make_in_maps(
        **inputs, upload_dt=build_kw.get("upload_dt", mybir.dt.float32)
    )
    res = run_bass_kernel_spmd(
        nc, in_maps, list(range(NCORES)), trace=trace, tmpdir=tmpdir
    )
    Y = np.concatenate([r["Y"] for r in res.results], axis=0)
    return Y, res


PROD_KW = dict(
    mode="fp8",
    span=(8, 8, 16),
    chunk=2,
    upload_dt=mybir.dt.bfloat16,
    stx_bufs=8,
    stw_bufs=6,
    xb_bufs=16,  # whole-K resident: DMA free-runs ahead of the PE
    wb_bufs=16,
    acc_dt=mybir.dt.float16,
    x_on_dve=False,
    copy_on_act=False,
    w_split=1024,
)


def kernel(**inputs) -> np.ndarray:
    Y, _ = run_on_cores(inputs, **PROD_KW)
    return Y
